# revision 99
# baseline (speedup 1.0000x reference)
"""Trainium2 Bass kernel for the masked-softmax attention module.

Computation (per batch row b):
    m      = lrelu(values[b] @ Wv.T + bv) + lrelu(query[b] @ Wq.T + bq)   [L, A]
    logit  = lrelu(tanh(m) @ Wo.T + bo)                                    [L]
    logit  = where(mask[b] == 0, -1e-9, logit)
    prob   = softmax(logit)
    out[b] = prob @ values[b]                                              [D]

Key algebraic cut: the reference masks with -1e-9 (not -inf), and
exp(-1e-9) rounds to exactly 1.0f in fp32.  Masked rows therefore
contribute weight exactly 1.0 to the softmax irrespective of their
logit, so their entire m / tanh / logit pipeline can be skipped.  The
host permutes each batch's rows so unmasked rows come first, the device
computes logits only for the first n1 rows (mask-fixup handles rows
between the true unmasked count and n1), the remaining L - n1 weights
are exactly 1.0 (memset for the partial l-tile, an all-ones stationary
column for fully-masked l-tiles), and the softmax sum is
(ACT accum over n1) + float(L - n1).  prob @ values runs over all L
permuted rows (the sum is permutation-invariant), so the result is
exactly the reference computation.

Load balancing: each core sorts its 4 batches by unmasked count into
slots; slot s is compiled with n1s[s] = max over cores of the s-th
largest count (rounded up to 32), so every slot is as tight as the
worst core requires but no tighter.

Sharding: data-parallel over batch, 4 batches per core on 8 NeuronCores.
All FLOPs run on device; the host only reshapes/casts/shards inputs.

DMA queues: sync carries qt/bq + the vt stream (prefetched one chunk
ahead); vector carries the wv stream; gpsimd the wq stream; scalar the
small consts, vn bulk loads and the output writes.
"""

import os
import sys

if "/opt/trn_rl_repo" not in sys.path:
    sys.path.insert(0, "/opt/trn_rl_repo")

import numpy as np
import ml_dtypes

from contextlib import ExitStack

import concourse.bass as bass
import concourse.tile as tile
from concourse import bacc, mybir
from concourse import bass_utils

BF = ml_dtypes.bfloat16
F32d = mybir.dt.float32
BF16d = mybir.dt.bfloat16
AF = mybir.ActivationFunctionType

NCORES = 8
B, L, D, A = 32, 1024, 2048, 2048
BL = B // NCORES          # batches per core
KD = D // 128             # d tiles
KA = A // 128             # a tiles
JL = L // 128             # l tiles
ALPHA = 0.01              # leaky relu slope
GRAN = 8                  # l-compaction granule


def _chunks(n1):
    """Split n1 into near-equal chunks of <=512 (PSUM f32 free-dim limit).

    Equal splits keep every matmul's moving stream longer than the
    ~128-row stationary load so LDWEIGHTS stays hidden."""
    k = (n1 + 511) // 512
    w0 = (n1 // k) // GRAN * GRAN
    widths = [w0] * k
    widths[-1] = n1 - w0 * (k - 1)
    out = []
    c0 = 0
    for w in widths:
        out.append((c0, w))
        c0 += w
    return out


def _slot_order(n1s):
    """Processing order: smallest first chunk first (fastest PE start),
    a multi-chunk slot last (shortest exposed softmax tail)."""
    ch_w0 = {s: _chunks(n1s[s])[0][1] for s in range(len(n1s))}
    fst = min(range(len(n1s)), key=lambda s: (ch_w0[s], s))
    rest = [s for s in range(len(n1s)) if s != fst]
    multi_rest = [s for s in rest if len(_chunks(n1s[s])) > 1]
    if multi_rest:
        lst = multi_rest[-1]
        rest.remove(lst)
        return [fst] + rest + [lst]
    return [fst] + rest


def build_graph(n1s, lrelu_mode="act"):
    """Build the per-core Bass graph (identical on all cores)."""
    nc = bacc.Bacc("TRN2", target_bir_lowering=False, debug=False)
    ntot = sum(n1s)
    offs = [sum(n1s[:s]) for s in range(BL)]
    t1s = [(n1 + 127) // 128 for n1 in n1s]
    t1max = max(t1s)
    n1max = max(n1s)

    # chunk-major flat layout: each chunk's [KD, cw] block is contiguous
    # per partition (~16 KB DMA runs instead of cw*2-byte ones)
    vt = nc.dram_tensor("vt", [128, KD * ntot], BF16d, kind="ExternalInput")
    vn = nc.dram_tensor("vn", [BL, 128, JL, D], BF16d, kind="ExternalInput")
    vtf = nc.dram_tensor("vtf", [BL, 128, KD, L], BF16d, kind="ExternalInput")
    wv = nc.dram_tensor("wv", [128, KA, KD, 128], BF16d, kind="ExternalInput")
    wq = nc.dram_tensor("wq", [128, KA, KD, 128], BF16d, kind="ExternalInput")
    qt = nc.dram_tensor("qt", [128, KD, BL], BF16d, kind="ExternalInput")
    wo = nc.dram_tensor("wo", [128, KA], BF16d, kind="ExternalInput")
    bvt = nc.dram_tensor("bvt", [128, KA], F32d, kind="ExternalInput")
    bqt = nc.dram_tensor("bqt", [128, KA], F32d, kind="ExternalInput")
    bo = nc.dram_tensor("bo", [1, 1], F32d, kind="ExternalInput")
    mf = nc.dram_tensor("mf", [1, ntot], BF16d, kind="ExternalInput")
    madd = nc.dram_tensor("madd", [1, ntot], BF16d, kind="ExternalInput")
    id4d = nc.dram_tensor("id4", [JL, JL], F32d, kind="ExternalInput")
    out = nc.dram_tensor("out", [BL, D], F32d, kind="ExternalOutput")
    # DVE-path output: outd[s, p, k] = out_row[128k+p]; host reassembles
    outd = nc.dram_tensor("outd", [BL, 128, KD], F32d, kind="ExternalOutput")

    def act_lrelu(out_ap, in_ap, bias_ap, pool, shape):
        if lrelu_mode == "act":
            nc.scalar.activation(out_ap, in_ap, AF.Lrelu, bias=bias_ap, alpha=ALPHA)
        else:
            # DVE fallback: lrelu(x + b) = max(x + b, ALPHA * (x + b))
            t1 = pool.tile(shape, F32d, tag="lr1")
            t2 = pool.tile(shape, F32d, tag="lr2")
            nc.vector.tensor_scalar(t1[:], in_ap, bias_ap, None, mybir.AluOpType.add)
            nc.vector.tensor_scalar(
                t2[:], in_ap, bias_ap, ALPHA, mybir.AluOpType.add, mybir.AluOpType.mult
            )
            nc.vector.tensor_max(out_ap, t1[:], t2[:])

    with tile.TileContext(nc) as tc, ExitStack() as ctx:
        const = ctx.enter_context(tc.tile_pool(name="const", bufs=1))
        wvp = ctx.enter_context(tc.tile_pool(name="wvp", bufs=1))

        # sync ring: qt/bq then the vt JIT stream
        qts_sb = const.tile([128, KD, BL], BF16d)
        nc.sync.dma_start(qts_sb[:], qt.ap()[:])
        bq_sb = const.tile([128, KA], F32d)
        nc.sync.dma_start(bq_sb[:], bqt.ap()[:])
        # scalar ring: small consts + vn bulk + output writes
        id8 = const.tile([JL, JL], F32d)
        nc.scalar.dma_start(id8[:], id4d.ap()[:])
        qp_sb = const.tile([128, KA, BL], F32d)
        wo_sb = const.tile([128, KA], BF16d)
        bv_sb = const.tile([128, KA], F32d)
        bo_sb = const.tile([1, 1], F32d)
        mf_sb = const.tile([1, ntot], BF16d)
        madd_sb = const.tile([1, ntot], BF16d)
        nc.scalar.dma_start(wo_sb[:], wo.ap()[:])
        nc.scalar.dma_start(bv_sb[:], bvt.ap()[:])
        nc.scalar.dma_start(bo_sb[:], bo.ap()[:])
        nc.scalar.dma_start(mf_sb[:], mf.ap()[:])
        nc.scalar.dma_start(madd_sb[:], madd.ap()[:])
        ones_sb = const.tile([128, 1], BF16d)
        nc.vector.memset(ones_sb[:], 1.0)

        # wv is laid out a-tile-major on its own (vector) ring
        wv_sb = wvp.tile([128, KA, KD, 128], BF16d)
        wv_loaded = set()

        def fetch_wv(j):
            # alternate rings so neither saturates during the first chunk
            if j < KA and j not in wv_loaded:
                eng = nc.sync if j % 2 == 0 else nc.scalar
                eng.dma_start(wv_sb[:, j, :, :], wv.ap()[:, j, :, :])
                wv_loaded.add(j)

        # q-projection is interleaved into slot 0 / chunk 0 of the main loop
        # (one group per GEMM group); wq streams on the gpsimd ring.
        wqp = ctx.enter_context(tc.tile_pool(name="wqp", bufs=3))
        psqp = ctx.enter_context(tc.tile_pool(name="psq", bufs=1, space="PSUM"))
        lrq = ctx.enter_context(tc.tile_pool(name="lrq", bufs=2))
        wq_tiles = {}

        def fetch_wq(t):
            if t < KA and t not in wq_tiles:
                wq_t = wqp.tile([128, KD, 128], BF16d)
                eng = nc.sync if t % 2 == 0 else nc.scalar
                eng.dma_start(wq_t[:], wq.ap()[:, t, :, :])
                wq_tiles[t] = wq_t

        def qproj_group(t):
            wq_t = wq_tiles.pop(t)
            psq = psqp.tile([128, BL], F32d)
            for k in range(KD):
                nc.tensor.matmul(
                    psq[:], lhsT=wq_t[:, k, :], rhs=qts_sb[:, k, :],
                    start=(k == 0), stop=(k == KD - 1),
                )
            act_lrelu(qp_sb[:, t, :], psq[:], bq_sb[:, t : t + 1], lrq, [128, BL])

        # PE warmup: dummy matmuls on zeroed tiles while the first DMAs land,
        # so the HAM clock gate is released before real work starts.
        wu_l = const.tile([128, 128], BF16d)
        nc.vector.memset(wu_l[:], 0.0)
        wu_ps = psqp.tile([128, 128], F32d, tag="psq")
        for i in range(52):
            nc.tensor.matmul(wu_ps[:], lhsT=wu_l[:], rhs=wu_l[:], start=(i == 0), stop=(i == 51))

        # ---- main loop ----
        vtp = ctx.enter_context(tc.tile_pool(name="vtp", bufs=2))
        vnp = ctx.enter_context(tc.tile_pool(name="vnp", bufs=1))
        vtfp = ctx.enter_context(tc.tile_pool(name="vtfp", bufs=1))
        s1p = ctx.enter_context(tc.tile_pool(name="s1p", bufs=2))
        thp = ctx.enter_context(tc.tile_pool(name="thp", bufs=17))
        smp = ctx.enter_context(tc.tile_pool(name="smp", bufs=1))
        outp = ctx.enter_context(tc.tile_pool(name="outp", bufs=2))
        psm = ctx.enter_context(tc.tile_pool(name="psm", bufs=2, space="PSUM"))
        psl = ctx.enter_context(tc.tile_pool(name="psl", bufs=1, space="PSUM"))
        pst = ctx.enter_context(tc.tile_pool(name="pst", bufs=1, space="PSUM"))
        pso = ctx.enter_context(tc.tile_pool(name="pso", bufs=2, space="PSUM"))

        sloop = _slot_order(n1s)
        fin_s = sloop[-1]

        # flat (slot, chunk) schedule with the vt stream prefetched one
        # chunk ahead so the next load overlaps the current compute and
        # never queues behind tail work on the ring
        sched = []
        for s in sloop:
            for ci, (c0, cw) in enumerate(_chunks(n1s[s])):
                sched.append((s, ci, c0, cw))
        vt_tiles = {}

        def issue_vt(i):
            if i < len(sched):
                s, ci, c0, cw = sched[i]
                t = vtp.tile([128, KD, cw], BF16d)
                o2 = KD * (offs[s] + c0)
                if i == 0:
                    # split the startup-critical first load across both
                    # rings (clean split along k: the flat layout is k-major)
                    kh = KD // 2
                    nc.sync.dma_start(
                        t[:, :kh, :], vt.ap()[:, o2 : o2 + kh * cw]
                    )
                    nc.scalar.dma_start(
                        t[:, kh:, :], vt.ap()[:, o2 + kh * cw : o2 + KD * cw]
                    )
                else:
                    nc.sync.dma_start(t[:], vt.ap()[:, o2 : o2 + KD * cw])
                vt_tiles[i] = t

        slot_state = {}
        pending_pe = []
        for i, (s, ci, c0, cw) in enumerate(sched):
            n1 = n1s[s]
            t1 = t1s[s]
            chunks_s = _chunks(n1)
            first = ci == 0
            last = ci == len(chunks_s) - 1
            fin = s == fin_s
            if first:
                logit_sb = smp.tile([1, n1max], F32d, tag="logit")
                p_f = smp.tile([1, L], F32d, tag="pf")
                slot_state[s] = [logit_sb, p_f, None, None, [0], None]
                mend = t1 * 128 if fin else L
                if n1 < mend:
                    nc.vector.memset(p_f[:, n1 : mend], 1.0)
                if not fin and i > 0:
                    # full-L d-major values for the DVE p@V path, issued a
                    # slot ahead of use (scalar: the sync ring carries the
                    # latency-critical vt/wv streams)
                    vtf_b = vtfp.tile([128, KD, L], BF16d)
                    nc.scalar.dma_start(vtf_b[:], vtf.ap()[s, :, :, :])
                    slot_state[s][5] = vtf_b
            logit_sb, p_f, ps_t, pT, tdone, vtf_b = slot_state[s][:6]
            vn_wait = None
            vtf_wait = None
            if fin and first:
                # natural-orientation values load (needed at slot end); for
                # the very first body defer the issue past the weight-stream
                # fetches so it doesn't block them in the ring FIFO
                vn_b = vnp.tile([128, JL, D], BF16d)
                if i == 0:
                    vn_wait = vn_b
                else:
                    nc.scalar.dma_start(vn_b[:], vn.ap()[s, :, :, :])
                slot_state[s].append(vn_b)
            if not fin and first and i == 0:
                vtf_b = vtfp.tile([128, KD, L], BF16d)
                vtf_wait = vtf_b
                slot_state[s][5] = vtf_b
            if fin:
                vn_b = slot_state[s][6]
            if i == 0:
                fetch_wq(0)
                fetch_wv(0)
                fetch_wq(1)
                issue_vt(0)
            vt_c = vt_tiles.pop(i)
            ps_l = psl.tile([1, cw], F32d)
            pending = []
            for j in range(KA):
                if i == 0:
                    qproj_group(j)
                    fetch_wq(j + 2)
                    fetch_wv(j + 1)
                    fetch_wv(j + 2)
                ps_m = psm.tile([128, cw], F32d)
                for k in range(KD):
                    nc.tensor.matmul(
                        ps_m[:],
                        lhsT=wv_sb[:, j, k, :],
                        rhs=vt_c[:, k, :],
                        start=(k == 0),
                        stop=(k == KD - 1),
                    )
                s1 = s1p.tile([128, cw], F32d)
                act_lrelu(s1[:], ps_m[:], bv_sb[:, j : j + 1], s1p, [128, cw])
                th = thp.tile([128, cw], BF16d)
                nc.scalar.activation(th[:], s1[:], AF.Tanh, bias=qp_sb[:, j, s : s + 1])
                pending.append((j, th))
            for x, (pj, pth) in enumerate(pending):
                nc.tensor.matmul(
                    ps_l[:], lhsT=wo_sb[:, pj : pj + 1], rhs=pth[:],
                    start=(pj == 0), stop=(x == len(pending) - 1),
                )
            pending = []
            issue_vt(i + 1)
            if vn_wait is not None:
                nc.scalar.dma_start(vn_wait[:], vn.ap()[s, :, :, :])
            if vtf_wait is not None:
                nc.scalar.dma_start(vtf_wait[:], vtf.ap()[s, :, :, :])
            lsl = logit_sb[:, c0 : c0 + cw]
            if lrelu_mode == "act":
                nc.scalar.activation(
                    lsl, ps_l[:], AF.Lrelu, bias=bo_sb[0:1, 0:1], alpha=ALPHA
                )
            else:
                act_lrelu(lsl, ps_l[:], bo_sb[0:1, 0:1], smp, [1, cw])
            # mask fixup + exp of this chunk (off the end-of-slot critical
            # path).  exp must NOT run on ACT: no activation table set holds
            # both exp and tanh, so each exp would force two ~1.3us table
            # reloads that stall the tanh->logit-matmul pipeline.  Instead
            # e^x = (1+tanh(x/2))/(1-tanh(x/2)): tanh stays in-table and the
            # rational part runs on the mostly-idle DVE.  (logits here are
            # O(4) so tanh(x/2) stays well away from 1.)
            o = offs[s] + c0
            nc.vector.tensor_mul(lsl, lsl, mf_sb[:, o : o + cw])
            nc.vector.tensor_add(lsl, lsl, madd_sb[:, o : o + cw])
            te = smp.tile([1, 512], F32d, tag="te")
            nc.scalar.activation(te[:, :cw], lsl, AF.Tanh, scale=0.5)
            num = smp.tile([1, 512], F32d, tag="tnum")
            nc.vector.tensor_scalar_add(num[:, :cw], te[:, :cw], 1.0)
            den = smp.tile([1, 512], F32d, tag="tden")
            nc.vector.tensor_scalar(
                den[:, :cw], te[:, :cw], -1.0, 1.0,
                mybir.AluOpType.mult, mybir.AluOpType.add,
            )
            rden = smp.tile([1, 512], F32d, tag="trden")
            nc.vector.reciprocal_approx_fast(rden[:, :cw], den[:, :cw])
            nc.vector.tensor_mul(p_f[:, c0 : c0 + cw], num[:, :cw], rden[:, :cw])
            if fin:
                # transpose every l-tile fully covered so far (PE cost ~0:
                # the [1,128] stationary load only) so the tail has no DMA
                tnew = t1 if last else (c0 + cw) // 128
                if tnew > tdone[0]:
                    if ps_t is None:
                        ps_t = pst.tile([128, t1max], F32d)
                        pT = smp.tile([128, t1max], BF16d, tag="pT")
                        slot_state[s][2] = ps_t
                        slot_state[s][3] = pT
                    for t in range(tdone[0], tnew):
                        nc.tensor.transpose(
                            ps_t[:, t : t + 1], p_f[:, 128 * t : 128 * t + 128],
                            id8[0:1, 0:1],
                        )
                    nc.vector.tensor_copy(
                        pT[:, tdone[0] : tnew], ps_t[:, tdone[0] : tnew]
                    )
                    tdone[0] = tnew

            if not last:
                continue

            if not fin:
                # ---- DVE p@V path: runs in the shadow of the next slot's
                # main GEMM; no PE work, no transposes, no vn load ----
                pcast = smp.tile([1, L], BF16d, tag="pcast")
                zs = smp.tile([1, 1], F32d, tag="zs")
                nc.vector.tensor_scalar(
                    pcast[:], p_f[:], 1.0, 0.0, mybir.AluOpType.mult,
                    mybir.AluOpType.add, accum_out=zs[:],
                )
                pb = smp.tile([128, L], BF16d, tag="pb")
                nc.gpsimd.partition_broadcast(pb[:], pcast[:])
                rs = smp.tile([1, 1], F32d, tag="rs")
                nc.vector.reciprocal(rs[:], zs[:])
                rs128 = smp.tile([128, 1], F32d, tag="rs128")
                nc.gpsimd.partition_broadcast(rs128[:], rs[:])
                scr = smp.tile([128, L], F32d, tag="scr")
                ov = smp.tile([128, KD], F32d, tag="ov")
                for k in range(KD):
                    nc.vector.scalar_tensor_tensor(
                        scr[:], vtf_b[:, k, :], 1.0, pb[:],
                        mybir.AluOpType.mult, mybir.AluOpType.mult,
                        accum_out=ov[:, k : k + 1],
                    )
                ovs = smp.tile([128, KD], F32d, tag="ovs")
                nc.vector.tensor_scalar_mul(ovs[:], ov[:], rs128[:, 0:1])
                nc.scalar.dma_start(outd.ap()[s, :, :], ovs[:])
                continue

            # ---- final slot tail (PE path): Z via a near-free PE
            # ones-matmul over the weight columns (fully-masked tiles
            # contribute exactly 128 each), then out[s] = (p @ values) / Z
            ps_z = pst.tile([1, 1], F32d, tag="z")
            for t in range(JL):
                lhs = pT[:, t : t + 1] if t < t1 else ones_sb[:, 0:1]
                nc.tensor.matmul(
                    ps_z[:], lhsT=lhs, rhs=ones_sb[:, 0:1],
                    start=(t == 0), stop=(t == JL - 1),
                )
            rs = smp.tile([1, 1], F32d, tag="rs")
            nc.vector.reciprocal(rs[:], ps_z[:])  # [1,1]: exact variant is cheap

            for dc in range(4):
                ps_o = pso.tile([1, 512], F32d)
                for t in range(JL):
                    lhs = pT[:, t : t + 1] if t < t1 else ones_sb[:, 0:1]
                    nc.tensor.matmul(
                        ps_o[:], lhsT=lhs,
                        rhs=vn_b[:, t, 512 * dc : 512 * dc + 512],
                        start=(t == 0), stop=(t == JL - 1),
                    )
                osl = outp.tile([1, 512], F32d)
                nc.vector.tensor_scalar_mul(osl[:], ps_o[:], rs[0:1, 0:1])
                nc.scalar.dma_start(
                    out.ap()[s : s + 1, 512 * dc : 512 * dc + 512], osl[:]
                )

    nc.compile()
    return nc


def prep_inputs(n1s, orders, query, values, mask, Wq, bq, Wv, bv, Wo, bo):
    """Host-side shard + layout prep. Returns list of 8 in_maps."""
    ntot = sum(n1s)
    offs = [sum(n1s[:s]) for s in range(BL)]
    Wv32 = np.ascontiguousarray(Wv, np.float32)
    Wq32 = np.ascontiguousarray(Wq, np.float32)
    # wv[p, j, k, i] = Wv[128j+i, 128k+p]  (WvT, a-tile-major chunks)
    wv_t = np.ascontiguousarray(
        Wv32.reshape(KA, 128, KD, 128).transpose(3, 0, 2, 1)
    ).astype(BF)
    # wq[p, t, k, i] = Wq[128t+i, 128k+p]  (WqT, a-tile-major chunks)
    wq_t = np.ascontiguousarray(
        Wq32.reshape(KA, 128, KD, 128).transpose(3, 0, 2, 1)
    ).astype(BF)
    wo_t = np.ascontiguousarray(Wo.reshape(KA, 128).T).astype(BF)
    bv_t = np.ascontiguousarray(bv.reshape(KA, 128).T).astype(np.float32)
    bq_t = np.ascontiguousarray(bq.reshape(KA, 128).T).astype(np.float32)
    bo_r = np.asarray(bo, np.float32).reshape(1, 1)

    sloop = _slot_order(n1s)
    fin_s = sloop[-1]
    in_maps = []
    for i in range(NCORES):
        order = orders[i]
        vt_i = np.zeros((128, KD * ntot), BF)
        vn_i = np.zeros((BL, 128, JL, D), BF)
        vtf_i = np.zeros((BL, 128, KD, L), BF)
        mf_i = np.zeros((1, ntot), BF)
        madd_i = np.zeros((1, ntot), BF)
        qsl = np.empty((BL, D), np.float32)
        for s in range(BL):
            bb = BL * i + order[s]
            n1 = n1s[s]
            off = offs[s]
            m = np.asarray(mask[bb])
            idx1 = np.flatnonzero(m != 0)
            idx0 = np.flatnonzero(m == 0)
            perm = np.concatenate([idx1, idx0])
            vp = np.asarray(values[bb], np.float32)[perm]
            # vpd[p, k, l] = vp[l, 128k+p] (d-major, full L)
            vpd = vp.T.reshape(KD, 128, L).transpose(1, 0, 2).astype(BF)
            # vt: compacted chunk-major flat blocks for the main GEMM
            for c0, cw in _chunks(n1):
                o2 = KD * (off + c0)
                vt_i[:, o2 : o2 + KD * cw] = vpd[:, :, c0 : c0 + cw].reshape(
                    128, KD * cw
                )
            if s == fin_s:
                # vn[s, p, j, d] = vp[128j+p, d] (final slot: PE p@V)
                vn_i[s] = vp.reshape(JL, 128, D).transpose(1, 0, 2).astype(BF)
            else:
                vtf_i[s] = vpd
            mperm = m[perm][:n1]
            mf_i[0, off : off + n1] = (mperm != 0).astype(BF)
            madd_i[0, off : off + n1] = ((mperm == 0) * np.float32(-1e-9)).astype(BF)
            qsl[s] = np.asarray(query[bb], np.float32)
        # qt[p, k, s] = qsl[s, 128k+p]
        qt_i = np.ascontiguousarray(
            qsl.T.reshape(KD, 128, BL).transpose(1, 0, 2)
        ).astype(BF)
        in_maps.append(
            {
                "vt": vt_i, "vn": vn_i, "vtf": vtf_i, "wv": wv_t, "wq": wq_t,
                "qt": qt_i, "wo": wo_t, "bvt": bv_t, "bqt": bq_t, "bo": bo_r,
                "mf": mf_i, "madd": madd_i, "id4": np.eye(JL, dtype=np.float32),
            }
        )
    return in_maps


_NC_CACHE = {}


def get_graph(n1s, lrelu_mode="act"):
    key = (tuple(n1s), lrelu_mode)
    if key not in _NC_CACHE:
        _NC_CACHE[key] = build_graph(list(n1s), lrelu_mode)
    return _NC_CACHE[key]


def plan(mask):
    """Per-slot n1 sizes and per-core batch->slot assignment."""
    counts = np.asarray(np.asarray(mask) != 0).sum(axis=1)
    orders = []
    slot_counts = np.zeros((NCORES, BL), np.int64)
    for i in range(NCORES):
        local = counts[BL * i : BL * (i + 1)]
        order = np.argsort(-local, kind="stable")
        orders.append(order)
        slot_counts[i] = local[order]
    n1s = []
    for s in range(BL):
        nmax = int(slot_counts[:, s].max())
        n1 = ((max(nmax, 1) + GRAN - 1) // GRAN) * GRAN
        n1s.append(min(n1, L))
    return n1s, orders


def run(inputs, trace=False, lrelu_mode="act"):
    n1s, orders = plan(inputs["mask"])
    nc = get_graph(n1s, lrelu_mode)
    in_maps = prep_inputs(n1s, orders, **inputs)
    res = bass_utils.run_bass_kernel_spmd(
        nc, in_maps, core_ids=list(range(NCORES)), trace=trace
    )
    fin_s = _slot_order(n1s)[-1]
    out = np.empty((B, D), np.float32)
    for i in range(NCORES):
        o = res.results[i]["out"]
        od = res.results[i]["outd"]
        for s in range(BL):
            if s == fin_s:
                row = o[s]
            else:
                # outd[s, p, k] = out_row[128k+p]
                row = np.ascontiguousarray(od[s].T).reshape(D)
            out[BL * i + orders[i][s]] = row
    return out, res


def kernel(**inputs):
    out, _ = run(inputs, trace=False)
    return out


# revision 101
# speedup vs baseline: 1.1760x; 1.1760x over previous
"""Trainium2 Bass kernel for the masked-softmax attention module.

Computation (per batch row b):
    m      = lrelu(values[b] @ Wv.T + bv) + lrelu(query[b] @ Wq.T + bq)   [L, A]
    logit  = lrelu(tanh(m) @ Wo.T + bo)                                    [L]
    logit  = where(mask[b] == 0, -1e-9, logit)
    prob   = softmax(logit)
    out[b] = prob @ values[b]                                              [D]

Key algebraic cut: the reference masks with -1e-9 (not -inf), and
exp(-1e-9) rounds to exactly 1.0f in fp32.  Masked rows therefore
contribute weight exactly 1.0 to the softmax irrespective of their
logit, so their entire m / tanh / logit pipeline can be skipped.  The
host permutes each batch's rows so unmasked rows come first, the device
computes logits only for the first n1 rows (mask-fixup handles rows
between the true unmasked count and n1), the remaining L - n1 weights
are exactly 1.0 (memset for the partial l-tile, an all-ones stationary
column for fully-masked l-tiles), and the softmax sum is
(ACT accum over n1) + float(L - n1).  prob @ values runs over all L
permuted rows (the sum is permutation-invariant), so the result is
exactly the reference computation.

Load balancing: each core sorts its 4 batches by unmasked count into
slots; slot s is compiled with n1s[s] = max over cores of the s-th
largest count (rounded up to 32), so every slot is as tight as the
worst core requires but no tighter.

Sharding: data-parallel over batch, 4 batches per core on 8 NeuronCores.
All FLOPs run on device; the host only reshapes/casts/shards inputs.

DMA queues: sync carries qt/bq + the vt stream (prefetched one chunk
ahead); vector carries the wv stream; gpsimd the wq stream; scalar the
small consts, vn bulk loads and the output writes.
"""

import os
import sys

if "/opt/trn_rl_repo" not in sys.path:
    sys.path.insert(0, "/opt/trn_rl_repo")

import numpy as np
import ml_dtypes

from contextlib import ExitStack

import concourse.bass as bass
import concourse.tile as tile
from concourse import bacc, mybir
from concourse import bass_utils

BF = ml_dtypes.bfloat16
F32d = mybir.dt.float32
BF16d = mybir.dt.bfloat16
AF = mybir.ActivationFunctionType

NCORES = 8
B, L, D, A = 32, 1024, 2048, 2048
BL = B // NCORES          # batches per core
KD = D // 128             # d tiles
KA = A // 128             # a tiles
JL = L // 128             # l tiles
ALPHA = 0.01              # leaky relu slope
GRAN = 8                  # l-compaction granule


def _chunks(n1):
    """Split n1 into near-equal chunks of <=512 (PSUM f32 free-dim limit).

    Equal splits keep every matmul's moving stream longer than the
    ~128-row stationary load so LDWEIGHTS stays hidden."""
    k = (n1 + 511) // 512
    w0 = (n1 // k) // GRAN * GRAN
    widths = [w0] * k
    widths[-1] = n1 - w0 * (k - 1)
    out = []
    c0 = 0
    for w in widths:
        out.append((c0, w))
        c0 += w
    return out


def _slot_order(n1s):
    """Processing order: smallest first chunk first (fastest PE start),
    a multi-chunk slot last (shortest exposed softmax tail)."""
    ch_w0 = {s: _chunks(n1s[s])[0][1] for s in range(len(n1s))}
    fst = min(range(len(n1s)), key=lambda s: (ch_w0[s], s))
    rest = [s for s in range(len(n1s)) if s != fst]
    multi_rest = [s for s in rest if len(_chunks(n1s[s])) > 1]
    if multi_rest:
        lst = multi_rest[-1]
        rest.remove(lst)
        return [fst] + rest + [lst]
    return [fst] + rest


def build_graph(n1s, lrelu_mode="act"):
    """Build the per-core Bass graph (identical on all cores)."""
    nc = bacc.Bacc("TRN2", target_bir_lowering=False, debug=False)
    ntot = sum(n1s)
    offs = [sum(n1s[:s]) for s in range(BL)]
    t1s = [(n1 + 127) // 128 for n1 in n1s]
    t1max = max(t1s)
    n1max = max(n1s)

    # chunk-major flat layout: each chunk's [KD, cw] block is contiguous
    # per partition (~16 KB DMA runs instead of cw*2-byte ones)
    vt = nc.dram_tensor("vt", [128, KD * ntot], BF16d, kind="ExternalInput")
    vn = nc.dram_tensor("vn", [BL, 128, JL, D], BF16d, kind="ExternalInput")
    vtf = nc.dram_tensor("vtf", [BL, 128, KD, L], BF16d, kind="ExternalInput")
    wv = nc.dram_tensor("wv", [128, KA, KD, 128], BF16d, kind="ExternalInput")
    wq = nc.dram_tensor("wq", [128, KA, KD, 128], BF16d, kind="ExternalInput")
    qt = nc.dram_tensor("qt", [128, KD, BL], BF16d, kind="ExternalInput")
    wo = nc.dram_tensor("wo", [128, KA], BF16d, kind="ExternalInput")
    bvt = nc.dram_tensor("bvt", [128, KA], F32d, kind="ExternalInput")
    bqt = nc.dram_tensor("bqt", [128, KA], F32d, kind="ExternalInput")
    bo = nc.dram_tensor("bo", [1, 1], F32d, kind="ExternalInput")
    mf = nc.dram_tensor("mf", [1, ntot], BF16d, kind="ExternalInput")
    madd = nc.dram_tensor("madd", [1, ntot], BF16d, kind="ExternalInput")
    id4d = nc.dram_tensor("id4", [JL, JL], F32d, kind="ExternalInput")
    out = nc.dram_tensor("out", [BL, D], F32d, kind="ExternalOutput")
    # DVE-path output: outd[s, p, k] = out_row[128k+p]; host reassembles
    outd = nc.dram_tensor("outd", [BL, 128, KD], F32d, kind="ExternalOutput")

    def act_lrelu(out_ap, in_ap, bias_ap, pool, shape):
        if lrelu_mode == "act":
            nc.scalar.activation(out_ap, in_ap, AF.Lrelu, bias=bias_ap, alpha=ALPHA)
        else:
            # DVE fallback: lrelu(x + b) = max(x + b, ALPHA * (x + b))
            t1 = pool.tile(shape, F32d, tag="lr1")
            t2 = pool.tile(shape, F32d, tag="lr2")
            nc.vector.tensor_scalar(t1[:], in_ap, bias_ap, None, mybir.AluOpType.add)
            nc.vector.tensor_scalar(
                t2[:], in_ap, bias_ap, ALPHA, mybir.AluOpType.add, mybir.AluOpType.mult
            )
            nc.vector.tensor_max(out_ap, t1[:], t2[:])

    with tile.TileContext(nc) as tc, ExitStack() as ctx:
        const = ctx.enter_context(tc.tile_pool(name="const", bufs=1))
        wvp = ctx.enter_context(tc.tile_pool(name="wvp", bufs=1))

        # sync ring: qt/bq then the vt JIT stream
        qts_sb = const.tile([128, KD, BL], BF16d)
        nc.sync.dma_start(qts_sb[:], qt.ap()[:])
        bq_sb = const.tile([128, KA], F32d)
        nc.sync.dma_start(bq_sb[:], bqt.ap()[:])
        # scalar ring: small consts + vn bulk + output writes
        id8 = const.tile([JL, JL], F32d)
        nc.scalar.dma_start(id8[:], id4d.ap()[:])
        qp_sb = const.tile([128, KA, BL], F32d)
        wo_sb = const.tile([128, KA], BF16d)
        bv_sb = const.tile([128, KA], F32d)
        bo_sb = const.tile([1, 1], F32d)
        mf_sb = const.tile([1, ntot], BF16d)
        madd_sb = const.tile([1, ntot], BF16d)
        nc.scalar.dma_start(wo_sb[:], wo.ap()[:])
        nc.scalar.dma_start(bv_sb[:], bvt.ap()[:])
        nc.scalar.dma_start(bo_sb[:], bo.ap()[:])
        nc.scalar.dma_start(mf_sb[:], mf.ap()[:])
        nc.scalar.dma_start(madd_sb[:], madd.ap()[:])
        ones_sb = const.tile([128, 1], BF16d)
        nc.vector.memset(ones_sb[:], 1.0)

        # wv is laid out a-tile-major on its own (vector) ring
        wv_sb = wvp.tile([128, KA, KD, 128], BF16d)
        wv_loaded = set()

        def fetch_wv(j):
            # alternate rings so neither saturates during the first chunk
            if j < KA and j not in wv_loaded:
                eng = nc.sync if j % 2 == 0 else nc.scalar
                eng.dma_start(wv_sb[:, j, :, :], wv.ap()[:, j, :, :])
                wv_loaded.add(j)

        # q-projection is interleaved into slot 0 / chunk 0 of the main loop
        # (one group per GEMM group); wq streams on the gpsimd ring.
        wqp = ctx.enter_context(tc.tile_pool(name="wqp", bufs=3))
        psqp = ctx.enter_context(tc.tile_pool(name="psq", bufs=1, space="PSUM"))
        lrq = ctx.enter_context(tc.tile_pool(name="lrq", bufs=2))
        wq_tiles = {}

        def fetch_wq(t):
            if t < KA and t not in wq_tiles:
                wq_t = wqp.tile([128, KD, 128], BF16d)
                eng = nc.sync if t % 2 == 0 else nc.scalar
                eng.dma_start(wq_t[:], wq.ap()[:, t, :, :])
                wq_tiles[t] = wq_t

        def qproj_group(t):
            wq_t = wq_tiles.pop(t)
            psq = psqp.tile([128, BL], F32d)
            for k in range(KD):
                nc.tensor.matmul(
                    psq[:], lhsT=wq_t[:, k, :], rhs=qts_sb[:, k, :],
                    start=(k == 0), stop=(k == KD - 1),
                )
            act_lrelu(qp_sb[:, t, :], psq[:], bq_sb[:, t : t + 1], lrq, [128, BL])

        # PE warmup: dummy matmuls on zeroed tiles while the first DMAs land,
        # so the HAM clock gate is released before real work starts.
        wu_l = const.tile([128, 128], BF16d)
        nc.vector.memset(wu_l[:], 0.0)
        wu_ps = psqp.tile([128, 128], F32d, tag="psq")
        for i in range(52):
            nc.tensor.matmul(wu_ps[:], lhsT=wu_l[:], rhs=wu_l[:], start=(i == 0), stop=(i == 51))

        # ---- main loop ----
        vtp = ctx.enter_context(tc.tile_pool(name="vtp", bufs=2))
        vnp = ctx.enter_context(tc.tile_pool(name="vnp", bufs=1))
        vtfp = ctx.enter_context(tc.tile_pool(name="vtfp", bufs=1))
        s1p = ctx.enter_context(tc.tile_pool(name="s1p", bufs=2))
        thp = ctx.enter_context(tc.tile_pool(name="thp", bufs=9))
        smp = ctx.enter_context(tc.tile_pool(name="smp", bufs=1))
        outp = ctx.enter_context(tc.tile_pool(name="outp", bufs=2))
        psm = ctx.enter_context(tc.tile_pool(name="psm", bufs=2, space="PSUM"))
        psl = ctx.enter_context(tc.tile_pool(name="psl", bufs=1, space="PSUM"))
        pst = ctx.enter_context(tc.tile_pool(name="pst", bufs=1, space="PSUM"))
        pso = ctx.enter_context(tc.tile_pool(name="pso", bufs=2, space="PSUM"))

        sloop = _slot_order(n1s)
        fin_s = sloop[-1]

        # flat (slot, chunk) schedule with the vt stream prefetched one
        # chunk ahead so the next load overlaps the current compute and
        # never queues behind tail work on the ring
        sched = []
        for s in sloop:
            for ci, (c0, cw) in enumerate(_chunks(n1s[s])):
                sched.append((s, ci, c0, cw))
        vt_tiles = {}

        def issue_vt(i):
            if i < len(sched):
                s, ci, c0, cw = sched[i]
                t = vtp.tile([128, KD, cw], BF16d)
                o2 = KD * (offs[s] + c0)
                if i == 0:
                    # split the startup-critical first load across both
                    # rings (clean split along k: the flat layout is k-major)
                    kh = KD // 2
                    nc.sync.dma_start(
                        t[:, :kh, :], vt.ap()[:, o2 : o2 + kh * cw]
                    )
                    nc.scalar.dma_start(
                        t[:, kh:, :], vt.ap()[:, o2 + kh * cw : o2 + KD * cw]
                    )
                else:
                    nc.sync.dma_start(t[:], vt.ap()[:, o2 : o2 + KD * cw])
                vt_tiles[i] = t

        slot_state = {}
        pending_pe = []
        for i, (s, ci, c0, cw) in enumerate(sched):
            n1 = n1s[s]
            t1 = t1s[s]
            chunks_s = _chunks(n1)
            first = ci == 0
            last = ci == len(chunks_s) - 1
            fin = s == fin_s
            if first:
                logit_sb = smp.tile([1, n1max], F32d, tag="logit")
                p_f = smp.tile([1, L], F32d, tag="pf")
                slot_state[s] = [logit_sb, p_f, None, None, [0], None]
                mend = t1 * 128 if fin else L
                if n1 < mend:
                    nc.vector.memset(p_f[:, n1 : mend], 1.0)
                if not fin and i > 0:
                    # full-L d-major values for the DVE p@V path, issued a
                    # slot ahead of use (scalar: the sync ring carries the
                    # latency-critical vt/wv streams)
                    vtf_b = vtfp.tile([128, KD, L], BF16d)
                    nc.scalar.dma_start(vtf_b[:], vtf.ap()[s, :, :, :])
                    slot_state[s][5] = vtf_b
            logit_sb, p_f, ps_t, pT, tdone, vtf_b = slot_state[s][:6]
            vn_wait = None
            vtf_wait = None
            if fin and first:
                # natural-orientation values load (needed at slot end); for
                # the very first body defer the issue past the weight-stream
                # fetches so it doesn't block them in the ring FIFO
                vn_b = vnp.tile([128, JL, D], BF16d)
                if i == 0:
                    vn_wait = vn_b
                else:
                    nc.scalar.dma_start(vn_b[:], vn.ap()[s, :, :, :])
                slot_state[s].append(vn_b)
            if not fin and first and i == 0:
                vtf_b = vtfp.tile([128, KD, L], BF16d)
                vtf_wait = vtf_b
                slot_state[s][5] = vtf_b
            if fin:
                vn_b = slot_state[s][6]
            if i == 0:
                fetch_wq(0)
                fetch_wv(0)
                fetch_wq(1)
                issue_vt(0)
            vt_c = vt_tiles.pop(i)
            ps_l = psl.tile([1, cw], F32d)
            pending = []
            for j in range(KA):
                if i == 0:
                    qproj_group(j)
                    # wv fetches ahead of wq in the ring FIFO: the main
                    # GEMM needs wv(j) just-in-time while qproj trails the
                    # wq stream by ~20 us of slack
                    fetch_wv(j + 1)
                    fetch_wv(j + 2)
                    fetch_wq(j + 2)
                ps_m = psm.tile([128, cw], F32d)
                for k in range(KD):
                    nc.tensor.matmul(
                        ps_m[:],
                        lhsT=wv_sb[:, j, k, :],
                        rhs=vt_c[:, k, :],
                        start=(k == 0),
                        stop=(k == KD - 1),
                    )
                s1 = s1p.tile([128, cw], F32d)
                act_lrelu(s1[:], ps_m[:], bv_sb[:, j : j + 1], s1p, [128, cw])
                th = thp.tile([128, cw], BF16d)
                nc.scalar.activation(th[:], s1[:], AF.Tanh, bias=qp_sb[:, j, s : s + 1])
                pending.append((j, th))
                if len(pending) == 9:
                    # emit the eight oldest logit matvecs back-to-back (their
                    # tanhs are >=1 group old): an eighth of the PSUM-group
                    # transitions of one-per-group emission
                    for _ in range(8):
                        pj, pth = pending.pop(0)
                        nc.tensor.matmul(
                            ps_l[:], lhsT=wo_sb[:, pj : pj + 1], rhs=pth[:],
                            start=(pj == 0), stop=False,
                        )
            for x, (pj, pth) in enumerate(pending):
                nc.tensor.matmul(
                    ps_l[:], lhsT=wo_sb[:, pj : pj + 1], rhs=pth[:],
                    start=(pj == 0), stop=(x == len(pending) - 1),
                )
            pending = []
            issue_vt(i + 1)
            if vn_wait is not None:
                nc.scalar.dma_start(vn_wait[:], vn.ap()[s, :, :, :])
            if vtf_wait is not None:
                nc.scalar.dma_start(vtf_wait[:], vtf.ap()[s, :, :, :])
            lsl = logit_sb[:, c0 : c0 + cw]
            if lrelu_mode == "act":
                nc.scalar.activation(
                    lsl, ps_l[:], AF.Lrelu, bias=bo_sb[0:1, 0:1], alpha=ALPHA
                )
            else:
                act_lrelu(lsl, ps_l[:], bo_sb[0:1, 0:1], smp, [1, cw])
            # mask fixup + exp of this chunk (off the end-of-slot critical
            # path).  exp must NOT run on ACT: no activation table set holds
            # both exp and tanh, so each exp would force two ~1.3us table
            # reloads that stall the tanh->logit-matmul pipeline.  Instead
            # e^x = (1+tanh(x/2))/(1-tanh(x/2)): tanh stays in-table and the
            # rational part runs on the mostly-idle DVE.  (logits here are
            # O(4) so tanh(x/2) stays well away from 1.)
            o = offs[s] + c0
            nc.vector.tensor_mul(lsl, lsl, mf_sb[:, o : o + cw])
            nc.vector.tensor_add(lsl, lsl, madd_sb[:, o : o + cw])
            te = smp.tile([1, 512], F32d, tag="te")
            nc.scalar.activation(te[:, :cw], lsl, AF.Tanh, scale=0.5)
            num = smp.tile([1, 512], F32d, tag="tnum")
            nc.vector.tensor_scalar_add(num[:, :cw], te[:, :cw], 1.0)
            den = smp.tile([1, 512], F32d, tag="tden")
            nc.vector.tensor_scalar(
                den[:, :cw], te[:, :cw], -1.0, 1.0,
                mybir.AluOpType.mult, mybir.AluOpType.add,
            )
            rden = smp.tile([1, 512], F32d, tag="trden")
            nc.vector.reciprocal_approx_fast(rden[:, :cw], den[:, :cw])
            nc.vector.tensor_mul(p_f[:, c0 : c0 + cw], num[:, :cw], rden[:, :cw])
            if fin:
                # transpose every l-tile fully covered so far (PE cost ~0:
                # the [1,128] stationary load only) so the tail has no DMA
                tnew = t1 if last else (c0 + cw) // 128
                if tnew > tdone[0]:
                    if ps_t is None:
                        ps_t = pst.tile([128, t1max], F32d)
                        pT = smp.tile([128, t1max], BF16d, tag="pT")
                        slot_state[s][2] = ps_t
                        slot_state[s][3] = pT
                    for t in range(tdone[0], tnew):
                        nc.tensor.transpose(
                            ps_t[:, t : t + 1], p_f[:, 128 * t : 128 * t + 128],
                            id8[0:1, 0:1],
                        )
                    nc.vector.tensor_copy(
                        pT[:, tdone[0] : tnew], ps_t[:, tdone[0] : tnew]
                    )
                    tdone[0] = tnew

            if not last:
                continue

            if not fin:
                # ---- DVE p@V path: runs in the shadow of the next slot's
                # main GEMM; no PE work, no transposes, no vn load ----
                pcast = smp.tile([1, L], BF16d, tag="pcast")
                zs = smp.tile([1, 1], F32d, tag="zs")
                nc.vector.tensor_scalar(
                    pcast[:], p_f[:], 1.0, 0.0, mybir.AluOpType.mult,
                    mybir.AluOpType.add, accum_out=zs[:],
                )
                pb = smp.tile([128, L], BF16d, tag="pb")
                nc.gpsimd.partition_broadcast(pb[:], pcast[:])
                rs = smp.tile([1, 1], F32d, tag="rs")
                nc.vector.reciprocal(rs[:], zs[:])
                rs128 = smp.tile([128, 1], F32d, tag="rs128")
                nc.gpsimd.partition_broadcast(rs128[:], rs[:])
                scr = smp.tile([128, L], F32d, tag="scr")
                ov = smp.tile([128, KD], F32d, tag="ov")
                for k in range(KD):
                    nc.vector.scalar_tensor_tensor(
                        scr[:], vtf_b[:, k, :], 1.0, pb[:],
                        mybir.AluOpType.mult, mybir.AluOpType.mult,
                        accum_out=ov[:, k : k + 1],
                    )
                ovs = smp.tile([128, KD], F32d, tag="ovs")
                nc.vector.tensor_scalar_mul(ovs[:], ov[:], rs128[:, 0:1])
                nc.scalar.dma_start(outd.ap()[s, :, :], ovs[:])
                continue

            # ---- final slot tail (PE path): Z via a near-free PE
            # ones-matmul over the weight columns (fully-masked tiles
            # contribute exactly 128 each), then out[s] = (p @ values) / Z
            ps_z = pst.tile([1, 1], F32d, tag="z")
            for t in range(JL):
                lhs = pT[:, t : t + 1] if t < t1 else ones_sb[:, 0:1]
                nc.tensor.matmul(
                    ps_z[:], lhsT=lhs, rhs=ones_sb[:, 0:1],
                    start=(t == 0), stop=(t == JL - 1),
                )
            rs = smp.tile([1, 1], F32d, tag="rs")
            nc.vector.reciprocal(rs[:], ps_z[:])  # [1,1]: exact variant is cheap

            for dc in range(4):
                ps_o = pso.tile([1, 512], F32d)
                for t in range(JL):
                    lhs = pT[:, t : t + 1] if t < t1 else ones_sb[:, 0:1]
                    nc.tensor.matmul(
                        ps_o[:], lhsT=lhs,
                        rhs=vn_b[:, t, 512 * dc : 512 * dc + 512],
                        start=(t == 0), stop=(t == JL - 1),
                    )
                osl = outp.tile([1, 512], F32d)
                nc.vector.tensor_scalar_mul(osl[:], ps_o[:], rs[0:1, 0:1])
                nc.scalar.dma_start(
                    out.ap()[s : s + 1, 512 * dc : 512 * dc + 512], osl[:]
                )

    nc.compile()
    return nc


def prep_inputs(n1s, orders, query, values, mask, Wq, bq, Wv, bv, Wo, bo):
    """Host-side shard + layout prep. Returns list of 8 in_maps."""
    ntot = sum(n1s)
    offs = [sum(n1s[:s]) for s in range(BL)]
    Wv32 = np.ascontiguousarray(Wv, np.float32)
    Wq32 = np.ascontiguousarray(Wq, np.float32)
    # wv[p, j, k, i] = Wv[128j+i, 128k+p]  (WvT, a-tile-major chunks)
    wv_t = np.ascontiguousarray(
        Wv32.reshape(KA, 128, KD, 128).transpose(3, 0, 2, 1)
    ).astype(BF)
    # wq[p, t, k, i] = Wq[128t+i, 128k+p]  (WqT, a-tile-major chunks)
    wq_t = np.ascontiguousarray(
        Wq32.reshape(KA, 128, KD, 128).transpose(3, 0, 2, 1)
    ).astype(BF)
    wo_t = np.ascontiguousarray(Wo.reshape(KA, 128).T).astype(BF)
    bv_t = np.ascontiguousarray(bv.reshape(KA, 128).T).astype(np.float32)
    bq_t = np.ascontiguousarray(bq.reshape(KA, 128).T).astype(np.float32)
    bo_r = np.asarray(bo, np.float32).reshape(1, 1)

    sloop = _slot_order(n1s)
    fin_s = sloop[-1]
    in_maps = []
    for i in range(NCORES):
        order = orders[i]
        vt_i = np.zeros((128, KD * ntot), BF)
        vn_i = np.zeros((BL, 128, JL, D), BF)
        vtf_i = np.zeros((BL, 128, KD, L), BF)
        mf_i = np.zeros((1, ntot), BF)
        madd_i = np.zeros((1, ntot), BF)
        qsl = np.empty((BL, D), np.float32)
        for s in range(BL):
            bb = BL * i + order[s]
            n1 = n1s[s]
            off = offs[s]
            m = np.asarray(mask[bb])
            idx1 = np.flatnonzero(m != 0)
            idx0 = np.flatnonzero(m == 0)
            perm = np.concatenate([idx1, idx0])
            vp = np.asarray(values[bb], np.float32)[perm]
            # vpd[p, k, l] = vp[l, 128k+p] (d-major, full L)
            vpd = vp.T.reshape(KD, 128, L).transpose(1, 0, 2).astype(BF)
            # vt: compacted chunk-major flat blocks for the main GEMM
            for c0, cw in _chunks(n1):
                o2 = KD * (off + c0)
                vt_i[:, o2 : o2 + KD * cw] = vpd[:, :, c0 : c0 + cw].reshape(
                    128, KD * cw
                )
            if s == fin_s:
                # vn[s, p, j, d] = vp[128j+p, d] (final slot: PE p@V)
                vn_i[s] = vp.reshape(JL, 128, D).transpose(1, 0, 2).astype(BF)
            else:
                vtf_i[s] = vpd
            mperm = m[perm][:n1]
            mf_i[0, off : off + n1] = (mperm != 0).astype(BF)
            madd_i[0, off : off + n1] = ((mperm == 0) * np.float32(-1e-9)).astype(BF)
            qsl[s] = np.asarray(query[bb], np.float32)
        # qt[p, k, s] = qsl[s, 128k+p]
        qt_i = np.ascontiguousarray(
            qsl.T.reshape(KD, 128, BL).transpose(1, 0, 2)
        ).astype(BF)
        in_maps.append(
            {
                "vt": vt_i, "vn": vn_i, "vtf": vtf_i, "wv": wv_t, "wq": wq_t,
                "qt": qt_i, "wo": wo_t, "bvt": bv_t, "bqt": bq_t, "bo": bo_r,
                "mf": mf_i, "madd": madd_i, "id4": np.eye(JL, dtype=np.float32),
            }
        )
    return in_maps


_NC_CACHE = {}


def get_graph(n1s, lrelu_mode="act"):
    key = (tuple(n1s), lrelu_mode)
    if key not in _NC_CACHE:
        _NC_CACHE[key] = build_graph(list(n1s), lrelu_mode)
    return _NC_CACHE[key]


def plan(mask):
    """Per-slot n1 sizes and per-core batch->slot assignment."""
    counts = np.asarray(np.asarray(mask) != 0).sum(axis=1)
    orders = []
    slot_counts = np.zeros((NCORES, BL), np.int64)
    for i in range(NCORES):
        local = counts[BL * i : BL * (i + 1)]
        order = np.argsort(-local, kind="stable")
        orders.append(order)
        slot_counts[i] = local[order]
    n1s = []
    for s in range(BL):
        nmax = int(slot_counts[:, s].max())
        n1 = ((max(nmax, 1) + GRAN - 1) // GRAN) * GRAN
        n1s.append(min(n1, L))
    return n1s, orders


def run(inputs, trace=False, lrelu_mode="act"):
    n1s, orders = plan(inputs["mask"])
    nc = get_graph(n1s, lrelu_mode)
    in_maps = prep_inputs(n1s, orders, **inputs)
    res = bass_utils.run_bass_kernel_spmd(
        nc, in_maps, core_ids=list(range(NCORES)), trace=trace
    )
    fin_s = _slot_order(n1s)[-1]
    out = np.empty((B, D), np.float32)
    for i in range(NCORES):
        o = res.results[i]["out"]
        od = res.results[i]["outd"]
        for s in range(BL):
            if s == fin_s:
                row = o[s]
            else:
                # outd[s, p, k] = out_row[128k+p]
                row = np.ascontiguousarray(od[s].T).reshape(D)
            out[BL * i + orders[i][s]] = row
    return out, res


def kernel(**inputs):
    out, _ = run(inputs, trace=False)
    return out
